# revision 22
# baseline (speedup 1.0000x reference)
"""FlowNetC correlation (nn_Correlation_27797028340332) on 8 TRN2 NeuronCores.

out[b, dy*21+dx, y, x] = mean_c in1[b,c,y,x] * in2p[b,c,y+2*dy, x+2*dx]
with in2p = zero-pad(in2, 20) and (dy, dx) over a 21x21 stride-2 grid.

Strategy (per core; data-parallel over batch B=8):
  - Inputs are cast to fp16 on the host and DMA'd straight into resident
    SBUF tiles (no on-chip cast), halving input HBM traffic vs fp32.
    in2 is NOT padded: the matmul windows are clipped to valid source
    columns, so the x-pad never exists anywhere in the pipeline.
  - The per-pixel C=256 dot products are banded Gram matmuls on the
    TensorEngine, 4x column-tiled (tile t = x-block xb, parity par;
    partitions 32t): each tile streams only the 42 VALID in2 columns of
    its 52-col band window (the other 10 hit x<0 or x>=128 and are
    identically zero -- never computed, stored, copied or dumped; the
    host re-embeds the zeros).  Trimming cut PE streaming, PSUM, copy
    and dump volume ~19% and was worth ~33% end to end.
  - PSUM: per (y, dy-batch<=12) one 2KB bank holds [128, 42*bsz] fp32
    (12 dys/bank -> 2 batches per row instead of 3), allocated
    per-batch from an 8-buffer pool so up to 8 banks / 4 rows are in
    flight: matmuls never wait on evacuation, keeping the PE in its
    fast p-state (this alone was a 4x win over 2x3-bank double
    buffering).  Each column tile clears has_written on its own
    32-partition stripe via start=True on its ch0 matmul (the clear is
    per-written-partition, so concurrent tiles sharing a bank do not
    clobber each other; no dummy-matmul clears needed).
  - Evacuation: each bank is drained by a column-split pair of copies,
    DVE taking ~45% and ACT ~55% (proportional to their clocks), so the
    two engines finish together and per-bank latency halves.
  - The needed output band G[x, x+2k] (k=0..20) is a per-partition
    diagonal no engine can extract at line rate, so the 42-column Grams
    are cast to fp16 and dumped to DRAM; the shear is a numpy strided
    view on the host inside kernel().
"""

import numpy as np

B, C, H, W = 8, 256, 96, 128
PAD = 20
D = 21            # displacements per axis
CH = 2            # contraction chunks of 128
XB = 32           # stationary columns per PE tile
JW = 42           # valid moving/dump columns per tile: each tile's
                  # 52-col band window contains exactly 10 x-pad columns
                  # (source col <0 or >=128) whose Gram entries are
                  # identically zero -- they are never computed, stored,
                  # copied or dumped; the host fills those zeros.
NT = 4            # PE column tiles (2 x-blocks x 2 parities)
BSZ = 12          # dy's per PSUM bank (12*42 = 504 <= 512 fp32 bank)
ROWBLK = 8
N_CORES = 8


def _valid_dys(y):
    """dy' indices with in-range source row y2 = y + 2*dy' - 20."""
    return [d for d in range(D) if 0 <= y + 2 * d - PAD < H]


def _batches(n):
    """Chunks of BSZ with remainder tail: n=21 -> (12,9); n=11 -> (11,)."""
    out = [BSZ] * (n // BSZ)
    if n % BSZ:
        out.append(n % BSZ)
    return out


def _dump_layout():
    """Per-y (n_dy, element offset) layout of the dump tensor's free dim."""
    offs, off = [], 0
    for y in range(H):
        n = len(_valid_dys(y))
        offs.append((n, off))
        off += n * JW
    return offs, off


_NC_CACHE = {}


def _build(reps=1):
    import contextlib

    import concourse.bacc as bacc
    import concourse.tile as tile
    import concourse.tile_rust as tile_rust
    from concourse import mybir

    offs, total = _dump_layout()

    nc = bacc.Bacc("TRN2", target_bir_lowering=False, debug=False)
    in1_d = nc.dram_tensor("in1", [C, H, W], mybir.dt.float16,
                           kind="ExternalInput").ap()
    in2_d = nc.dram_tensor("in2", [C, H, W], mybir.dt.float16,
                           kind="ExternalInput").ap()
    dump_d = nc.dram_tensor("dump", [128, total], mybir.dt.float16,
                            kind="ExternalOutput").ap()

    YGRP = 4                      # y rows per dump DMA
    MAXW = D * JW                 # 882 elems: worst-case per-y dump width

    with tile.TileContext(nc) as tc:
        with tc.tile_pool(name="resident", bufs=1) as res_pool, \
             tc.tile_pool(name="out", bufs=4) as out_pool, \
             tc.tile_pool(name="psum", bufs=8, space="PSUM") as psum_pool, \
             (tc.For_i(0, reps, 1) if reps > 1 else contextlib.nullcontext()):

            # Fully-resident fp16 feature maps.  No x-padding: trimmed
            # moving windows never read outside [0, W), so there are no
            # pad memsets at all.
            in1s = res_pool.tile([128, CH, H, W], mybir.dt.float16)
            in2s = res_pool.tile([128, CH, H, W], mybir.dt.float16)

            def load1_block(yb, eng=None):
                y0 = yb * ROWBLK
                (eng or nc.gpsimd).dma_start(
                    in1s[:, :, y0:y0 + ROWBLK, :],
                    in1_d[:, y0:y0 + ROWBLK, :].rearrange(
                        "(k p) y x -> p k y x", p=128))

            def load2_block(yb, eng=None):
                y0 = yb * ROWBLK
                for k in range(CH):
                    (eng or nc.gpsimd).dma_start(
                        in2s[:, k, y0:y0 + ROWBLK, :],
                        in2_d[128 * k:128 * (k + 1), y0:y0 + ROWBLK, :]
                        .rearrange("p y x -> p y x"))

            def load_block(yb):
                """DMA rows [yb*8, yb*8+8) of both inputs into SBUF."""
                load1_block(yb)
                load2_block(yb)

            # Prologue on the idle HWDGE rings (SP + ACT), y=0's deps
            # first (in1 rows 0-7, in2 rows 0-23), then the rest of the
            # 4-block lookahead window; steady-state loads use SWDGE.
            # (Splitting the first in1 load to row-0-only starts y=0
            # ~1.1us earlier but starves y=1-4 — net worse; measured.)
            load1_block(0, nc.sync)
            load2_block(0, nc.scalar)
            load2_block(1, nc.sync)
            load2_block(2, nc.scalar)
            load2_block(3, nc.sync)
            for yb in range(1, 4):
                load1_block(yb)

            def copy_dve(out, in_):
                nc.vector.tensor_copy(out, in_)

            def copy_act(out, in_):
                nc.scalar.copy(out, in_)

            # Dump groups of 4 y's, except the last 8 y's in pairs so the
            # final copies+DMA tail after the last matmul stays short.
            gstart = set(range(0, H - 8, YGRP)) | set(range(H - 8, H, 2))
            gend = {y - 1 for y in gstart if y > 0} | {H - 1}

            stage = None
            goff = 0
            gbase = 0
            for y in range(H):
                # Stay 3-4 blocks ahead of the in2 read frontier (y+20).
                if y % ROWBLK == 0:
                    yb = y // ROWBLK + 4
                    if yb < H // ROWBLK:
                        load_block(yb)

                if y in gstart:
                    stage = out_pool.tile([128, YGRP * MAXW],
                                          mybir.dt.float16, tag="dumpstage")
                    goff = 0
                    gbase = offs[y][1]

                dys = _valid_dys(y)
                n_dy = len(dys)
                bs = _batches(n_dy)

                di = 0
                for ib, bsz in enumerate(bs):
                    # One PSUM bank per dy-batch, 6 in flight: matmuls for
                    # later batches never wait on this batch's evacuation,
                    # so the PE stays busy (and in its fast p-state) while
                    # DVE/ACT drain earlier banks.
                    ps = psum_pool.tile([128, 512], mybir.dt.float32,
                                        tag="ps")
                    dy0 = dys[di]
                    y2f = y + 2 * dy0 - PAD
                    # Dummy 1-col matmuls, one per column tile: start=True
                    # clears the bank's has_written bits (their union spans
                    # all 128 partitions); they write only col 511 (never
                    # read).  Real matmuls then use start=False so the 4
                    # concurrent column tiles can share the bank.  Col-tiled
                    # dummies chain behind their own group's stream instead
                    # of barriering the whole array like a 128-wide one.
                    # No dummies: each column tile clears has_written on
                    # its own 32-partition stripe with start=True on its ch0
                    # matmul (testing per-partition clear semantics).
                    prev = None
                    for ch in range(CH):
                        for t in range(NT):
                            xb, par = t // 2, t % 2
                            lo = 64 * xb + par
                            # First valid source col for this tile's band:
                            # xb=0 tiles clip the left pad (10 cols), xb=1
                            # tiles start at x=44+par and clip the right.
                            s0 = par if xb == 0 else 44 + par
                            mm = nc.tensor.matmul(
                                ps[XB * t:XB * (t + 1), 0:bsz * JW],
                                in1s[:, ch, y, lo:lo + 2 * XB - 1:2],
                                in2s[:, ch, y2f:y2f + 2 * bsz - 1:2,
                                     s0:s0 + 2 * JW - 1:2],
                                start=(ch == 0),
                                stop=(ch == CH - 1),
                                skip_group_check=True,
                                tile_position=(0, XB * t))
                            if prev is not None:
                                tile_rust.add_dep_helper(
                                    mm.ins, prev.ins, sync=False,
                                    reason="psum bank order")
                            prev = mm
                    # Per-batch evacuation, column-split DVE/ACT ~45/55
                    # (proportional to 0.96 vs 1.2 GHz) so both engines
                    # finish together and per-bank drain latency halves.
                    w = bsz * JW
                    wd = (w * 45 // 100) & ~3
                    ob = goff + di * JW
                    copy_dve(stage[:, ob:ob + wd], ps[:, 0:wd])
                    copy_act(stage[:, ob + wd:ob + w], ps[:, wd:w])
                    di += bsz

                goff += n_dy * JW

                if y in gend:
                    # Alternate the two HWDGE rings so consecutive dumps
                    # (and especially the epilogue pair) drain in parallel.
                    deng = nc.sync if (y // YGRP) % 2 == 0 else nc.scalar
                    deng.dma_start(dump_d[:, gbase:gbase + goff],
                                   stage[:, 0:goff])

    nc.compile()
    return nc, offs, total


def _get_nc():
    if "nc" not in _NC_CACHE:
        _NC_CACHE["nc"] = _build()
    return _NC_CACHE["nc"]


def _assemble(dump, offs):
    """Shear one core's fp16 Gram dump into [441, H, W] fp32.

    Each tile stripe stores the 42 valid band columns; the 10 trimmed
    columns (x-pad, identically zero) are re-embedded as zeros at the
    window offset (xb=0 -> [10,52), xb=1 -> [0,42)) before extracting
    the per-partition diagonal with a strided view.
    """
    JF = 52                                       # full band width
    out = np.zeros((D * D, H, W), np.float32)
    ks = np.arange(D)
    for y in range(H):
        n, off = offs[y]
        blk = np.ascontiguousarray(
            dump[:, off:off + n * JW]).astype(np.float32) / np.float32(C)
        blk = blk.reshape(128, n, JW)
        dys = np.array(_valid_dys(y))
        d_idx = (dys[:, None] * D + ks[None, :]).ravel()
        for t in range(NT):
            xb, par = t // 2, t % 2
            lo = JF - JW if xb == 0 else 0
            g = np.zeros((XB, n, JF), np.float32)
            g[:, :, lo:lo + JW] = blk[XB * t:XB * (t + 1)]
            s = g.strides
            diag = np.lib.stride_tricks.as_strided(
                g, shape=(n, D, XB), strides=(s[1], s[2], s[0] + s[2]))
            xsl = slice(64 * xb + par, 64 * xb + par + 2 * XB, 2)
            out[d_idx, y, xsl] = diag.reshape(n * D, XB)
    return out


def kernel(input1: np.ndarray, input2: np.ndarray) -> np.ndarray:
    from concourse.bass_utils import run_bass_kernel_spmd

    nc, offs, total = _get_nc()
    in_maps = [
        {"in1": np.ascontiguousarray(input1[b]).astype(np.float16),
         "in2": np.ascontiguousarray(input2[b]).astype(np.float16)}
        for b in range(N_CORES)
    ]
    res = run_bass_kernel_spmd(nc, in_maps, list(range(N_CORES)))
    out = np.empty((B, D * D, H, W), np.float32)
    for b in range(N_CORES):
        out[b] = _assemble(res.results[b]["dump"], offs)
    return out



# revision 23
# speedup vs baseline: 1.0840x; 1.0840x over previous
"""FlowNetC correlation (nn_Correlation_27797028340332) on 8 TRN2 NeuronCores.

out[b, dy*21+dx, y, x] = mean_c in1[b,c,y,x] * in2p[b,c,y+2*dy, x+2*dx]
with in2p = zero-pad(in2, 20) and (dy, dx) over a 21x21 stride-2 grid.

Strategy (per core; data-parallel over batch B=8):
  - Inputs are cast to fp16 on the host and DMA'd straight into resident
    SBUF tiles (no on-chip cast), halving input HBM traffic vs fp32.
    in2 is NOT padded: the matmul windows are clipped to valid source
    columns, so the x-pad never exists anywhere in the pipeline.
  - The per-pixel C=256 dot products are banded Gram matmuls on the
    TensorEngine, 4x column-tiled (tile t = x-block xb, parity par;
    partitions 32t): each tile streams only the 42 VALID in2 columns of
    its 52-col band window (the other 10 hit x<0 or x>=128 and are
    identically zero -- never computed, stored, copied or dumped; the
    host re-embeds the zeros).  Trimming cut PE streaming, PSUM, copy
    and dump volume ~19% and was worth ~33% end to end.
  - PSUM: per (y, dy-batch<=12) one 2KB bank holds [128, 42*bsz] fp32
    (12 dys/bank -> 2 batches per row instead of 3), allocated
    per-batch from an 8-buffer pool so up to 8 banks / 4 rows are in
    flight: matmuls never wait on evacuation, keeping the PE in its
    fast p-state (this alone was a 4x win over 2x3-bank double
    buffering).  Each column tile clears has_written on its own
    32-partition stripe via start=True on its ch0 matmul (the clear is
    per-written-partition, so concurrent tiles sharing a bank do not
    clobber each other; no dummy-matmul clears needed).
  - Evacuation: each bank is drained by a column-split pair of copies,
    DVE taking ~45% and ACT ~55% (proportional to their clocks), so the
    two engines finish together and per-bank latency halves.
  - The needed output band G[x, x+2k] (k=0..20) is a per-partition
    diagonal no engine can extract at line rate, so the 42-column Grams
    are cast to fp16 and dumped to DRAM; the shear is a numpy strided
    view on the host inside kernel().
"""

import numpy as np

B, C, H, W = 8, 256, 96, 128
PAD = 20
D = 21            # displacements per axis
CH = 2            # contraction chunks of 128
XB = 32           # stationary columns per PE tile
JW = 42           # valid moving/dump columns per tile: each tile's
                  # 52-col band window contains exactly 10 x-pad columns
                  # (source col <0 or >=128) whose Gram entries are
                  # identically zero -- they are never computed, stored,
                  # copied or dumped; the host fills those zeros.
NT = 4            # PE column tiles (2 x-blocks x 2 parities)
BSZ = 12          # dy's per PSUM bank (12*42 = 504 <= 512 fp32 bank)
ROWBLK = 8
N_CORES = 8


def _valid_dys(y):
    """dy' indices with in-range source row y2 = y + 2*dy' - 20."""
    return [d for d in range(D) if 0 <= y + 2 * d - PAD < H]


def _batches(n):
    """Chunks of BSZ with remainder tail: n=21 -> (12,9); n=11 -> (11,)."""
    out = [BSZ] * (n // BSZ)
    if n % BSZ:
        out.append(n % BSZ)
    return out


def _dump_layout():
    """Per-y (n_dy, element offset) layout of the dump tensor's free dim."""
    offs, off = [], 0
    for y in range(H):
        n = len(_valid_dys(y))
        offs.append((n, off))
        off += n * JW
    return offs, off


_NC_CACHE = {}


def _build(reps=1):
    import contextlib

    import concourse.bacc as bacc
    import concourse.tile as tile
    import concourse.tile_rust as tile_rust
    from concourse import mybir

    offs, total = _dump_layout()

    nc = bacc.Bacc("TRN2", target_bir_lowering=False, debug=False)
    in1_d = nc.dram_tensor("in1", [C, H, W], mybir.dt.float16,
                           kind="ExternalInput").ap()
    in2_d = nc.dram_tensor("in2", [C, H, W], mybir.dt.float16,
                           kind="ExternalInput").ap()
    dump_d = nc.dram_tensor("dump", [128, total], mybir.dt.float16,
                            kind="ExternalOutput").ap()

    YGRP = 4                      # y rows per dump DMA
    MAXW = D * JW                 # 882 elems: worst-case per-y dump width

    with tile.TileContext(nc) as tc:
        with tc.tile_pool(name="resident", bufs=1) as res_pool, \
             tc.tile_pool(name="out", bufs=6) as out_pool, \
             tc.tile_pool(name="psum", bufs=8, space="PSUM") as psum_pool, \
             (tc.For_i(0, reps, 1) if reps > 1 else contextlib.nullcontext()):

            # Fully-resident fp16 feature maps.  No x-padding: trimmed
            # moving windows never read outside [0, W), so there are no
            # pad memsets at all.
            in1s = res_pool.tile([128, CH, H, W], mybir.dt.float16)
            in2s = res_pool.tile([128, CH, H, W], mybir.dt.float16)

            def load1_block(yb, eng=None):
                y0 = yb * ROWBLK
                (eng or nc.gpsimd).dma_start(
                    in1s[:, :, y0:y0 + ROWBLK, :],
                    in1_d[:, y0:y0 + ROWBLK, :].rearrange(
                        "(k p) y x -> p k y x", p=128))

            def load2_block(yb, eng=None):
                y0 = yb * ROWBLK
                for k in range(CH):
                    (eng or nc.gpsimd).dma_start(
                        in2s[:, k, y0:y0 + ROWBLK, :],
                        in2_d[128 * k:128 * (k + 1), y0:y0 + ROWBLK, :]
                        .rearrange("p y x -> p y x"))

            def load_block(yb):
                """DMA rows [yb*8, yb*8+8) of both inputs into SBUF."""
                load1_block(yb)
                load2_block(yb)

            # Prologue on the idle HWDGE rings (SP + ACT), y=0's deps
            # first (in1 rows 0-7, in2 rows 0-23), then the rest of the
            # 4-block lookahead window; steady-state loads use SWDGE.
            # (Splitting the first in1 load to row-0-only starts y=0
            # ~1.1us earlier but starves y=1-4 — net worse; measured.)
            load1_block(0, nc.sync)
            load2_block(0, nc.scalar)
            load2_block(1, nc.sync)
            load2_block(2, nc.scalar)
            load2_block(3, nc.sync)
            for yb in range(1, 4):
                load1_block(yb)

            def copy_dve(out, in_):
                nc.vector.tensor_copy(out, in_)

            def copy_act(out, in_):
                nc.scalar.copy(out, in_)

            # Dump groups of 4 y's, except the last 8 y's in pairs so the
            # final copies+DMA tail after the last matmul stays short.
            gstart = set(range(0, H - 8, YGRP)) | set(range(H - 8, H, 2))
            gend = {y - 1 for y in gstart if y > 0} | {H - 1}

            stage = None
            goff = 0
            gbase = 0
            for y in range(H):
                # Stay 3-4 blocks ahead of the in2 read frontier (y+20).
                if y % ROWBLK == 0:
                    yb = y // ROWBLK + 4
                    if yb < H // ROWBLK:
                        load_block(yb)

                if y in gstart:
                    stage = out_pool.tile([128, YGRP * MAXW],
                                          mybir.dt.float16, tag="dumpstage")
                    goff = 0
                    gbase = offs[y][1]

                dys = _valid_dys(y)
                n_dy = len(dys)
                bs = _batches(n_dy)

                di = 0
                for ib, bsz in enumerate(bs):
                    # One PSUM bank per dy-batch, 6 in flight: matmuls for
                    # later batches never wait on this batch's evacuation,
                    # so the PE stays busy (and in its fast p-state) while
                    # DVE/ACT drain earlier banks.
                    ps = psum_pool.tile([128, 512], mybir.dt.float32,
                                        tag="ps")
                    dy0 = dys[di]
                    y2f = y + 2 * dy0 - PAD
                    # Dummy 1-col matmuls, one per column tile: start=True
                    # clears the bank's has_written bits (their union spans
                    # all 128 partitions); they write only col 511 (never
                    # read).  Real matmuls then use start=False so the 4
                    # concurrent column tiles can share the bank.  Col-tiled
                    # dummies chain behind their own group's stream instead
                    # of barriering the whole array like a 128-wide one.
                    # No dummies: each column tile clears has_written on
                    # its own 32-partition stripe with start=True on its ch0
                    # matmul (testing per-partition clear semantics).
                    prev = None
                    for ch in range(CH):
                        for t in range(NT):
                            xb, par = t // 2, t % 2
                            lo = 64 * xb + par
                            # First valid source col for this tile's band:
                            # xb=0 tiles clip the left pad (10 cols), xb=1
                            # tiles start at x=44+par and clip the right.
                            s0 = par if xb == 0 else 44 + par
                            mm = nc.tensor.matmul(
                                ps[XB * t:XB * (t + 1), 0:bsz * JW],
                                in1s[:, ch, y, lo:lo + 2 * XB - 1:2],
                                in2s[:, ch, y2f:y2f + 2 * bsz - 1:2,
                                     s0:s0 + 2 * JW - 1:2],
                                start=(ch == 0),
                                stop=(ch == CH - 1),
                                skip_group_check=True,
                                tile_position=(0, XB * t))
                            if prev is not None:
                                tile_rust.add_dep_helper(
                                    mm.ins, prev.ins, sync=False,
                                    reason="psum bank order")
                            prev = mm
                    # Per-batch evacuation, column-split DVE/ACT ~45/55
                    # (proportional to 0.96 vs 1.2 GHz) so both engines
                    # finish together and per-bank drain latency halves.
                    w = bsz * JW
                    wd = (w * 45 // 100) & ~3
                    ob = goff + di * JW
                    copy_dve(stage[:, ob:ob + wd], ps[:, 0:wd])
                    copy_act(stage[:, ob + wd:ob + w], ps[:, wd:w])
                    di += bsz

                goff += n_dy * JW

                if y in gend:
                    # Alternate the two HWDGE rings so consecutive dumps
                    # (and especially the epilogue pair) drain in parallel.
                    deng = nc.sync if (y // YGRP) % 2 == 0 else nc.scalar
                    deng.dma_start(dump_d[:, gbase:gbase + goff],
                                   stage[:, 0:goff])

    nc.compile()
    return nc, offs, total


def _get_nc():
    if "nc" not in _NC_CACHE:
        _NC_CACHE["nc"] = _build()
    return _NC_CACHE["nc"]


def _assemble(dump, offs):
    """Shear one core's fp16 Gram dump into [441, H, W] fp32.

    Each tile stripe stores the 42 valid band columns; the 10 trimmed
    columns (x-pad, identically zero) are re-embedded as zeros at the
    window offset (xb=0 -> [10,52), xb=1 -> [0,42)) before extracting
    the per-partition diagonal with a strided view.
    """
    JF = 52                                       # full band width
    out = np.zeros((D * D, H, W), np.float32)
    ks = np.arange(D)
    for y in range(H):
        n, off = offs[y]
        blk = np.ascontiguousarray(
            dump[:, off:off + n * JW]).astype(np.float32) / np.float32(C)
        blk = blk.reshape(128, n, JW)
        dys = np.array(_valid_dys(y))
        d_idx = (dys[:, None] * D + ks[None, :]).ravel()
        for t in range(NT):
            xb, par = t // 2, t % 2
            lo = JF - JW if xb == 0 else 0
            g = np.zeros((XB, n, JF), np.float32)
            g[:, :, lo:lo + JW] = blk[XB * t:XB * (t + 1)]
            s = g.strides
            diag = np.lib.stride_tricks.as_strided(
                g, shape=(n, D, XB), strides=(s[1], s[2], s[0] + s[2]))
            xsl = slice(64 * xb + par, 64 * xb + par + 2 * XB, 2)
            out[d_idx, y, xsl] = diag.reshape(n * D, XB)
    return out


def kernel(input1: np.ndarray, input2: np.ndarray) -> np.ndarray:
    from concourse.bass_utils import run_bass_kernel_spmd

    nc, offs, total = _get_nc()
    in_maps = [
        {"in1": np.ascontiguousarray(input1[b]).astype(np.float16),
         "in2": np.ascontiguousarray(input2[b]).astype(np.float16)}
        for b in range(N_CORES)
    ]
    res = run_bass_kernel_spmd(nc, in_maps, list(range(N_CORES)))
    out = np.empty((B, D * D, H, W), np.float32)
    for b in range(N_CORES):
        out[b] = _assemble(res.results[b]["dump"], offs)
    return out

